# revision 1
# baseline (speedup 1.0000x reference)
"""Trainium2 Bass kernel for nn_ODE4: explicit-Euler neural ODE + MLP head.

  y_{t+1} = y_t + dt_t * (tanh([y_t, e_t] @ Wr1 + br1) @ Wr2 + br2)
  out     = relu(preds @ W1 + b1) @ W2 + b2          # preds = [y_0..y_{T-1}]

Sharding: pure data parallel over batch B across 8 cores (128 rows each);
tiny weights replicated; the sequential scan over T stays local per core.

On-chip layout is feature-major ([S|H, batch] on partitions) so the tiny
contractions run on the PE. All y_t / e_t slices live at partition base 0
(a PE requirement), free-dim packed: chunk tiles [8, TC*128], slot t at
free offset 128*t.

  per step:  psum_h  = Wy^T y_t + We^T e_t   (2 matmuls, K=8)
             h       = tanh(psum_h + br1)    (ACT, per-partition bias)
             psum_f  = Wr2^T h (+ br2)       (matmul, K=32)
             y_{t+1} = (psum_f * dt_t) + y_t (fused DVE scalar_tensor_tensor)

x arrives [B, T, E] batch-major; PE transposes ([128,8] -> [8,128] into a
free-packed PSUM bank) produce the e-slots, DVE copies them to SBUF.

Head (bulk, overlapped with the scan):
  pre1[10,B] = W1^T y_t            -> relu+bias b1 (DVE tensor_scalar)
  out[B,2]   = u_t^T @ W2  with u_t as the stationary operand, free-packed
               into a [128, 2*TC] PSUM tile => already [b,(t,c)] for the DMA.
"""

import numpy as np
from contextlib import ExitStack

import concourse.bass as bass
import concourse.bacc as bacc
import concourse.mybir as mybir
from concourse.tile import TileContext
from concourse import bass_utils

F32 = mybir.dt.float32
AF = mybir.ActivationFunctionType
ALU = mybir.AluOpType

B, T, S, E, H = 1024, 4096, 8, 8, 32
NCORES = 8
BC = B // NCORES  # 128 per-core batch rows = matmul free dim


def build_ode_nc(T=T, TC=64, with_br2=False):
    """Emit the per-core Bass program. All cores run the same code (SPMD)."""
    assert TC % 4 == 0 and T % TC == 0
    nchunks = T // TC

    nc = bacc.Bacc()
    xs_d = nc.dram_tensor("xs", [BC, T * E], F32, kind="ExternalInput")
    y0t_d = nc.dram_tensor("y0t", [S, BC], F32, kind="ExternalInput")
    dtb_d = nc.dram_tensor("dtb", [S, T], F32, kind="ExternalInput")
    wy_d = nc.dram_tensor("wy", [S, H], F32, kind="ExternalInput")
    we_d = nc.dram_tensor("we", [E, H], F32, kind="ExternalInput")
    wr2_d = nc.dram_tensor("wr2", [H, S], F32, kind="ExternalInput")
    br1_d = nc.dram_tensor("br1c", [H, 1], F32, kind="ExternalInput")
    w1_d = nc.dram_tensor("w1", [S, 10], F32, kind="ExternalInput")
    w2_d = nc.dram_tensor("w2", [10, 2], F32, kind="ExternalInput")
    ident_d = nc.dram_tensor("ident", [128, 128], F32, kind="ExternalInput")
    if with_br2:
        br2_d = nc.dram_tensor("br2r", [1, S], F32, kind="ExternalInput")
    b1_d = nc.dram_tensor("b1c", [10, 1], F32, kind="ExternalInput")
    out_d = nc.dram_tensor("out", [BC, T * 2], F32, kind="ExternalOutput")

    with TileContext(nc) as tc, ExitStack() as ctx:
        cpool = ctx.enter_context(tc.tile_pool(name="consts", bufs=1))
        xbp = ctx.enter_context(tc.tile_pool(name="xb", bufs=2))
        xep = ctx.enter_context(tc.tile_pool(name="xe", bufs=2))
        ysp = ctx.enter_context(tc.tile_pool(name="ys", bufs=2))
        hp = ctx.enter_context(tc.tile_pool(name="h", bufs=3))
        up = ctx.enter_context(tc.tile_pool(name="u", bufs=3))
        osbp = ctx.enter_context(tc.tile_pool(name="osb", bufs=2))
        psp = ctx.enter_context(tc.tile_pool(name="psp", bufs=2, space="PSUM"))
        pup = ctx.enter_context(tc.tile_pool(name="pup", bufs=2, space="PSUM"))
        ptp = ctx.enter_context(tc.tile_pool(name="ptp", bufs=2, space="PSUM"))
        pop = ctx.enter_context(tc.tile_pool(name="pop", bufs=2, space="PSUM"))

        def cload(name, shape, dram):
            t_ = cpool.tile(shape, F32, tag=name)
            nc.sync.dma_start(t_[:], dram[:])
            return t_

        wy_t = cload("wy", [S, H], wy_d)
        we_t = cload("we", [E, H], we_d)
        wr2_t = cload("wr2", [H, S], wr2_d)
        br1_t = cload("br1", [H, 1], br1_d)
        w1_t = cload("w1", [S, 10], w1_d)
        w2_t = cload("w2", [10, 2], w2_d)
        id_t = cload("ident", [128, 128], ident_d)
        dt_t = cload("dtb", [S, T], dtb_d)
        b1_t = cload("b1", [10, 1], b1_d)
        if with_br2:
            br2_t = cload("br2", [1, S], br2_d)
            ones_t = cpool.tile([1, 128], F32, tag="ones")
            nc.gpsimd.memset(ones_t[:], 1.0)

        ys_tiles = []

        def new_ys_tile():
            t_ = ysp.tile([S, TC * 128], F32, tag="ys")
            ys_tiles.append(t_)
            return t_

        def yslot(g):
            """AP of y_g: [8, 128] at free offset 128*(g%TC)."""
            c, s = divmod(g, TC)
            return ys_tiles[c][:, 128 * s:128 * (s + 1)]

        ys0 = new_ys_tile()
        nc.sync.dma_start(ys0[:, 0:128], y0t_d[:])

        for c in range(nchunks):
            # ---- PRE: load + transpose x chunk into free-packed e-slots ----
            xb_t = xbp.tile([128, TC * E], F32, tag="xb")
            nc.sync.dma_start(xb_t[:], xs_d[:, c * TC * E:(c + 1) * TC * E])
            xe_t = xep.tile([S, TC * 128], F32, tag="xe")
            for blk in range(TC // 4):
                ptile = ptp.tile([S, 512], F32, tag="pt", space="PSUM")
                for k in range(4):
                    s = 4 * blk + k
                    nc.tensor.transpose(ptile[:, 128 * k:128 * (k + 1)],
                                        xb_t[:, 8 * s:8 * s + 8], id_t[:])
                nc.vector.tensor_copy(xe_t[:, 512 * blk:512 * (blk + 1)],
                                      ptile[:])

            def eslot(s):
                return xe_t[:, 128 * s:128 * (s + 1)]

            # ---- SCAN over this chunk ----
            for s in range(TC):
                g = c * TC + s
                if g >= T - 1:
                    break
                if g + 1 >= len(ys_tiles) * TC:
                    new_ys_tile()
                ya = yslot(g)
                ph = psp.tile([H, 128], F32, tag="sp", space="PSUM")
                nc.tensor.matmul(ph[:], wy_t[:], ya, start=True, stop=False)
                nc.tensor.matmul(ph[:], we_t[:], eslot(s),
                                 start=False, stop=True)
                h_t = hp.tile([H, 128], F32, tag="h")
                nc.scalar.activation(h_t[:], ph[:], AF.Tanh, bias=br1_t[:])
                pf = psp.tile([S, 128], F32, tag="sp", space="PSUM")
                nc.tensor.matmul(pf[:], wr2_t[:], h_t[:], start=True,
                                 stop=not with_br2)
                if with_br2:
                    nc.tensor.matmul(pf[:], br2_t[:], ones_t[:],
                                     start=False, stop=True)
                nc.vector.scalar_tensor_tensor(
                    yslot(g + 1), pf[:], dt_t[:, g:g + 1], ya,
                    ALU.mult, ALU.add)

            # ---- POST: MLP head for all t in this chunk ----
            po = pop.tile([128, 2 * TC], F32, tag="po", space="PSUM")
            for q4 in range(TC // 4):
                pu_t = pup.tile([10, 512], F32, tag="pu", space="PSUM")
                for k in range(4):
                    s = 4 * q4 + k
                    nc.tensor.matmul(pu_t[:, 128 * k:128 * (k + 1)], w1_t[:],
                                     yslot(c * TC + s), start=True, stop=True)
                u_t = up.tile([10, 512], F32, tag="u")
                nc.vector.tensor_scalar(u_t[:], pu_t[:], b1_t[:], 0.0,
                                        ALU.add, ALU.max)
                for k in range(4):
                    s = 4 * q4 + k
                    nc.tensor.matmul(po[:, 2 * s:2 * s + 2],
                                     u_t[:, 128 * k:128 * (k + 1)], w2_t[:],
                                     start=True, stop=True)
            osb_t = osbp.tile([128, 2 * TC], F32, tag="osb")
            nc.vector.tensor_copy(osb_t[:], po[:])
            nc.sync.dma_start(out_d[:, 2 * c * TC:2 * (c + 1) * TC],
                              osb_t[:])

    nc.compile()
    return nc


def _prep_inputs(x, t, y0, Wr1, br1, Wr2, br2, W1, b1, W2, b2, T_=T):
    """Host-side: build per-core input maps."""
    x = np.ascontiguousarray(np.asarray(x, np.float32))
    dt = np.zeros((T_,), np.float32)
    dt[:T_ - 1] = np.diff(np.asarray(t, np.float32))
    dtb = np.broadcast_to(dt[None, :], (S, T_)).copy()
    Wr1 = np.asarray(Wr1, np.float32)
    common = {
        "dtb": dtb,
        "wy": np.ascontiguousarray(Wr1[:S]),
        "we": np.ascontiguousarray(Wr1[S:]),
        "wr2": np.ascontiguousarray(np.asarray(Wr2, np.float32)),
        "br1c": np.asarray(br1, np.float32).reshape(H, 1).copy(),
        "w1": np.ascontiguousarray(np.asarray(W1, np.float32)),
        "w2": np.ascontiguousarray(np.asarray(W2, np.float32)),
        "ident": np.eye(128, dtype=np.float32),
        "b1c": np.asarray(b1, np.float32).reshape(10, 1).copy(),
    }
    with_br2 = bool(np.any(np.asarray(br2) != 0))
    if with_br2:
        common["br2r"] = np.asarray(br2, np.float32).reshape(1, S).copy()
    y0 = np.asarray(y0, np.float32)
    in_maps = []
    for k in range(NCORES):
        sl = slice(k * BC, (k + 1) * BC)
        in_maps.append({
            "xs": x[sl].reshape(BC, T_ * E).copy(),
            "y0t": np.ascontiguousarray(y0[sl].T),
            **common,
        })
    return in_maps, with_br2


# ---------------------------------------------------------------------------
# v2: scan in pre-activation space. State p_t = Wy^T y_t + We^T e_t + br1
# lives in a persistent PSUM accumulator; each step is only
#   h = tanh(p)  (ACT) ;  p += dtW~^T h + We^T e_{t+1} - We^T e_t  (PE)
# so the serial chain is 2 hops (ACT -> PE -> ACT). p_t is copied out by DVE
# (off-chain) and the head consumes p via host-folded matrices:
#   pre1 = M1 p - (M1 We^T) e + (b1 - M1 br1),  M1 = W1^T pinv(Wy^T).
# ---------------------------------------------------------------------------


def build_ode_nc_v2(T=T, TC=32, with_br2=False):
    assert TC % 4 == 0 and T % TC == 0
    nchunks = T // TC

    nc = bacc.Bacc()
    xs_d = nc.dram_tensor("xs", [BC, T * E], F32, kind="ExternalInput")
    y0t_d = nc.dram_tensor("y0t", [S, BC], F32, kind="ExternalInput")
    dtw_d = nc.dram_tensor("dtw", [H, T * H], F32, kind="ExternalInput")
    wy_d = nc.dram_tensor("wy", [S, H], F32, kind="ExternalInput")
    we_d = nc.dram_tensor("we", [E, H], F32, kind="ExternalInput")
    wem_d = nc.dram_tensor("wem", [E, H], F32, kind="ExternalInput")
    br1r_d = nc.dram_tensor("br1r", [1, H], F32, kind="ExternalInput")
    atl_d = nc.dram_tensor("atl", [H, 10], F32, kind="ExternalInput")
    bml_d = nc.dram_tensor("bml", [E, 10], F32, kind="ExternalInput")
    btc_d = nc.dram_tensor("btc", [10, 1], F32, kind="ExternalInput")
    w2_d = nc.dram_tensor("w2", [10, 2], F32, kind="ExternalInput")
    ident_d = nc.dram_tensor("ident", [128, 128], F32, kind="ExternalInput")
    if with_br2:
        dtbr2_d = nc.dram_tensor("dtbr2", [1, T * H], F32,
                                 kind="ExternalInput")
    out_d = nc.dram_tensor("out", [BC, T * 2], F32, kind="ExternalOutput")

    with TileContext(nc) as tc, ExitStack() as ctx:
        cpool = ctx.enter_context(tc.tile_pool(name="consts", bufs=1))
        dbr2p = ctx.enter_context(tc.tile_pool(name="dbr2p", bufs=3))
        xbp = ctx.enter_context(tc.tile_pool(name="xb", bufs=3))
        xep = ctx.enter_context(tc.tile_pool(name="xe", bufs=3))
        psb = ctx.enter_context(tc.tile_pool(name="psb", bufs=2))
        dtwp = ctx.enter_context(tc.tile_pool(name="dtwp", bufs=3))
        hp = ctx.enter_context(tc.tile_pool(name="h", bufs=3))
        up = ctx.enter_context(tc.tile_pool(name="u", bufs=3))
        osbp = ctx.enter_context(tc.tile_pool(name="osb", bufs=2))
        ppp = ctx.enter_context(tc.tile_pool(name="ppp", bufs=1, space="PSUM"))
        pup = ctx.enter_context(tc.tile_pool(name="pup", bufs=2, space="PSUM"))
        ptp = ctx.enter_context(tc.tile_pool(name="ptp", bufs=2, space="PSUM"))
        pop = ctx.enter_context(tc.tile_pool(name="pop", bufs=2, space="PSUM"))

        def cload(name, shape, dram):
            t_ = cpool.tile(shape, F32, tag=name)
            nc.sync.dma_start(t_[:], dram[:])
            return t_

        wy_t = cload("wy", [S, H], wy_d)
        we_t = cload("we", [E, H], we_d)
        wem_t = cload("wem", [E, H], wem_d)
        br1r_t = cload("br1r", [1, H], br1r_d)
        atl_t = cload("atl", [H, 10], atl_d)
        bml_t = cload("bml", [E, 10], bml_d)
        btc_t = cload("btc", [10, 1], btc_d)
        w2_t = cload("w2", [10, 2], w2_d)
        id_t = cload("ident", [128, 128], ident_d)
        y0s_t = cload("y0s", [S, BC], y0t_d)
        ones_t = cpool.tile([1, 128], F32, tag="ones")
        nc.gpsimd.memset(ones_t[:], 1.0)

        pp_t = ppp.tile([H, 128], F32, tag="pp", name="pp", space="PSUM")

        xe_tiles, ps_tiles, dtw_tiles, dtbr2_tiles = [], [], [], []

        def pre(c):
            xb_t = xbp.tile([128, TC * E], F32, tag="xb")
            nc.sync.dma_start(xb_t[:], xs_d[:, c * TC * E:(c + 1) * TC * E])
            xe_t = xep.tile([S, TC * 128], F32, tag="xe")
            for blk in range(TC // 4):
                ptile = ptp.tile([S, 512], F32, tag="pt", space="PSUM")
                for k in range(4):
                    s = 4 * blk + k
                    nc.tensor.transpose(ptile[:, 128 * k:128 * (k + 1)],
                                        xb_t[:, 8 * s:8 * s + 8], id_t[:])
                nc.vector.tensor_copy(xe_t[:, 512 * blk:512 * (blk + 1)],
                                      ptile[:])
            xe_tiles.append(xe_t)
            dtw_t = dtwp.tile([H, TC * H], F32, tag="dtw")
            nc.sync.dma_start(dtw_t[:],
                              dtw_d[:, c * TC * H:(c + 1) * TC * H])
            dtw_tiles.append(dtw_t)
            if with_br2:
                db_t = dbr2p.tile([1, TC * H], F32, tag="dbr2")
                nc.sync.dma_start(db_t[:],
                                  dtbr2_d[:, c * TC * H:(c + 1) * TC * H])
                dtbr2_tiles.append(db_t)

        def eslot(g):
            c, s = divmod(g, TC)
            return xe_tiles[c][:, 128 * s:128 * (s + 1)]

        pre(0)
        # p_0 = Wy^T y0 + We^T e_0 + br1
        nc.tensor.matmul(pp_t[:], wy_t[:], y0s_t[:], start=True, stop=False,
                         skip_group_check=True)
        nc.tensor.matmul(pp_t[:], we_t[:], eslot(0), start=False, stop=False,
                         skip_group_check=True)
        nc.tensor.matmul(pp_t[:], br1r_t[:], ones_t[:],
                         start=False, stop=True, skip_group_check=True)

        for c in range(nchunks):
            if c + 1 < nchunks:
                pre(c + 1)
            ps_t = psb.tile([H, TC * 128], F32, tag="ps")
            ps_tiles.append(ps_t)

            # ---- SCAN ----
            for s in range(TC):
                g = c * TC + s
                nc.vector.tensor_copy(ps_t[:, 128 * s:128 * (s + 1)],
                                      pp_t[:])
                if g >= T - 1:
                    break
                h_t = hp.tile([H, 128], F32, tag="h")
                nc.scalar.activation(h_t[:], pp_t[:], AF.Tanh)
                nc.tensor.matmul(pp_t[:], we_t[:], eslot(g + 1),
                                 start=False, stop=False,
                                 skip_group_check=True)
                nc.tensor.matmul(pp_t[:], wem_t[:], eslot(g),
                                 start=False, stop=False,
                                 skip_group_check=True)
                if with_br2:
                    nc.tensor.matmul(pp_t[:],
                                     dtbr2_tiles[c][:, H * s:H * (s + 1)],
                                     ones_t[:], start=False, stop=False,
                                     skip_group_check=True)
                nc.tensor.matmul(pp_t[:],
                                 dtw_tiles[c][:, H * s:H * (s + 1)],
                                 h_t[:], start=False, stop=True,
                                 skip_group_check=True)

            # ---- POST: head from stored p and e ----
            po = pop.tile([128, 2 * TC], F32, tag="po", space="PSUM")
            for q4 in range(TC // 4):
                pu_t = pup.tile([10, 512], F32, tag="pu", space="PSUM")
                for k in range(4):
                    s = 4 * q4 + k
                    g = c * TC + s
                    nc.tensor.matmul(pu_t[:, 128 * k:128 * (k + 1)],
                                     atl_t[:], ps_t[:, 128 * s:128 * (s + 1)],
                                     start=True, stop=False)
                    nc.tensor.matmul(pu_t[:, 128 * k:128 * (k + 1)],
                                     bml_t[:], eslot(g),
                                     start=False, stop=True)
                u_t = up.tile([10, 512], F32, tag="u")
                nc.vector.tensor_scalar(u_t[:], pu_t[:], btc_t[:], 0.0,
                                        ALU.add, ALU.max)
                for k in range(4):
                    s = 4 * q4 + k
                    nc.tensor.matmul(po[:, 2 * s:2 * s + 2],
                                     u_t[:, 128 * k:128 * (k + 1)], w2_t[:],
                                     start=True, stop=True)
            osb_t = osbp.tile([128, 2 * TC], F32, tag="osb")
            nc.vector.tensor_copy(osb_t[:], po[:])
            nc.sync.dma_start(out_d[:, 2 * c * TC:2 * (c + 1) * TC],
                              osb_t[:])

    nc.compile()
    return nc


def _prep_inputs_v2(x, t, y0, Wr1, br1, Wr2, br2, W1, b1, W2, b2, T_=T):
    x = np.ascontiguousarray(np.asarray(x, np.float32))
    dt = np.zeros((T_,), np.float32)
    dt[:T_ - 1] = np.diff(np.asarray(t, np.float32))
    Wr1 = np.asarray(Wr1, np.float32)
    Wy, We = Wr1[:S], Wr1[S:]
    Wr2 = np.asarray(Wr2, np.float32)
    W1 = np.asarray(W1, np.float32)
    br1 = np.asarray(br1, np.float32)
    Wt = (Wr2 @ Wy).astype(np.float32)                     # [H, H]
    dtw = (Wt[:, None, :] * dt[None, :, None]).astype(np.float32)
    M1 = (W1.T @ np.linalg.pinv(Wy.T.astype(np.float64))).astype(np.float32)
    common = {
        "dtw": np.ascontiguousarray(dtw.reshape(H, T_ * H)),
        "wy": np.ascontiguousarray(Wy),
        "we": np.ascontiguousarray(We),
        "wem": np.ascontiguousarray(-We),
        "br1r": br1.reshape(1, H).copy(),
        "atl": np.ascontiguousarray(M1.T),                 # [H, 10]
        "bml": np.ascontiguousarray(-(We @ M1.T)),         # [E, 10]
        "btc": (np.asarray(b1, np.float32)
                - M1 @ br1).reshape(10, 1).copy(),
        "w2": np.ascontiguousarray(np.asarray(W2, np.float32)),
        "ident": np.eye(128, dtype=np.float32),
    }
    with_br2 = bool(np.any(np.asarray(br2) != 0))
    if with_br2:
        wyb = (Wy.T.astype(np.float32)
               @ np.asarray(br2, np.float32).reshape(S))   # [H]
        dtbr2 = (wyb[None, None, :] * dt[None, :, None]).astype(np.float32)
        common["dtbr2"] = np.ascontiguousarray(dtbr2.reshape(1, T_ * H))
    y0 = np.asarray(y0, np.float32)
    in_maps = []
    for k in range(NCORES):
        sl = slice(k * BC, (k + 1) * BC)
        in_maps.append({
            "xs": x[sl].reshape(BC, T_ * E).copy(),
            "y0t": np.ascontiguousarray(y0[sl].T),
            **common,
        })
    return in_maps, with_br2


_NC_CACHE = {}


def kernel(x, t, y0, Wr1, br1, Wr2, br2, W1, b1, W2, b2):
    in_maps, with_br2 = _prep_inputs_v2(
        x, t, y0, Wr1, br1, Wr2, br2, W1, b1, W2, b2)
    key = ("v2", with_br2)
    if key not in _NC_CACHE:
        _NC_CACHE[key] = build_ode_nc_v2(T=T, TC=32, with_br2=with_br2)
    nc = _NC_CACHE[key]
    res = bass_utils.run_bass_kernel_spmd(nc, in_maps,
                                          core_ids=list(range(NCORES)))
    outs = [res.results[k]["out"].reshape(BC, T, 2) for k in range(NCORES)]
    out = np.concatenate(outs, axis=0)
    b2 = np.asarray(b2, np.float32)
    if np.any(b2 != 0):
        out = out + b2[None, None, :]
    return out.astype(np.float32)

